# revision 3
# baseline (speedup 1.0000x reference)
"""Neural CDE encoder kernel for 8 Trainium2 NeuronCores.

Math (from the reference):
  - Natural cubic spline on unit-spaced knots; Euler times t_k = 0.05*k for
    k=0..19 all lie in interval [0,1), so only interval-0 coefficients matter:
        dX(t) = (y1 - y0) + M1 * (t^2/2 - 1/6)
    with M1 = <w, y> for a constant weight vector w over L (row 0 of the
    tridiagonal inverse, second-differenced).
  - Euler: z_{k+1} = z_k + dt * einsum('bhd,bd->bh', reshape(z W^T + b), dX_k)
  - Output: project grid z's with W_out, then linearly interpolate (linear ops
    commute) via a constant (L x 21) matrix.

Sharding: tensor-parallel over H. Each core owns a contiguous 96-row slice of
H (6144 rows of W_lin), kept resident in SBUF as W^T (768, 6144) f32. Per step
each core computes f_loc = z @ W_loc^T (64, 6144) on the PE (stationary = z^T
k-tiles, moving = W^T columns, fp32r), contracts over d on the DVE against
dX_k, updates its (96, 64) z-slice (h-major), and an AllGather rebuilds the
full z^T (768, 64). W_out is sharded over O (32 cols per core); grid
projections go to DRAM and a final constant interp matmul writes each core's
(64, 128, 32) output slice; the host concatenates along O.
"""

import numpy as np

B, L, D, H, O = 64, 128, 64, 768, 256
NS = 20            # Euler steps
NC = 8             # cores
HLOC = H // NC     # 96
HDLOC = HLOC * D   # 6144
OLOC = O // NC     # 32
KT = H // 128      # 6 contraction tiles
NT = HDLOC // 512  # 12 moving tiles per step
GRP = 3            # psum bank group size for f tiles

_prog_cache = {}


def _host_constants():
    # Euler grid, all in f32 to match the reference
    grid = (np.arange(NS + 1, dtype=np.float32) * np.float32(0.05)).astype(np.float32)
    grid[-1] = np.float32(1.0)
    dts = (grid[1:] - grid[:-1]).astype(np.float32)
    tk = grid[:-1].astype(np.float64)
    mcoef = (tk * tk / 2.0 - 1.0 / 6.0).astype(np.float32)

    # w over L such that M1 = <w, y>:  M_inner = A^{-1} rhs, rhs_j = 6*(y_{j+2}
    # - 2 y_{j+1} + y_j);  M1 = M_inner[0] = sum_j Ainv[0, j] rhs_j
    n = L - 2
    A = 4.0 * np.eye(n) + np.eye(n, k=1) + np.eye(n, k=-1)
    r0 = np.linalg.solve(A, np.eye(n)[:, 0])
    w = np.zeros(L, dtype=np.float64)
    w[0:n] += 6.0 * r0
    w[1:n + 1] += -12.0 * r0
    w[2:n + 2] += 6.0 * r0
    wG = np.zeros((L, 2), dtype=np.float32)
    wG[:, 0] = w.astype(np.float32)
    wG[1, 1] = 1.0
    wG[0, 1] = -1.0  # column 1 extracts base = y1 - y0

    # Interp matrix J (L, NS+1): out_z[l] = sum_k J[l,k] z_grid[k]
    ts = np.linspace(0.0, 1.0, L, dtype=np.float32)
    j = np.clip(np.searchsorted(grid, ts, side="right") - 1, 0, NS - 1)
    wl = ((ts - grid[j]) / (grid[j + 1] - grid[j])).astype(np.float32)
    J = np.zeros((L, NS + 1), dtype=np.float32)
    J[np.arange(L), j] += 1.0 - wl
    J[np.arange(L), j + 1] += wl
    return dts, mcoef, wG, J.T.copy()  # JT (21, 128)


def _build_program(dts, mcoef, has_blin, has_bout, ns=NS):
    import concourse.bacc as bacc
    import concourse.mybir as mybir
    import concourse.tile as tile

    f32 = mybir.dt.float32
    f32r = mybir.dt.float32r
    ADD = mybir.AluOpType.add
    MUL = mybir.AluOpType.mult

    nc = bacc.Bacc("TRN2", target_bir_lowering=False, debug=False, num_devices=NC)

    # ---- I/O -------------------------------------------------------------
    traj_d = nc.dram_tensor("traj", [B, L, D], f32, kind="ExternalInput")
    wt_d = nc.dram_tensor("wt_loc", [H, HDLOC], f32r, kind="ExternalInput")
    wz0_d = nc.dram_tensor("wz0_aug", [D + 1, H], f32, kind="ExternalInput")
    wz0l_d = nc.dram_tensor("wz0l_aug", [D + 1, HLOC], f32, kind="ExternalInput")
    wo_d = nc.dram_tensor("wo_loc", [H, OLOC], f32r, kind="ExternalInput")
    wg_d = nc.dram_tensor("wg", [L, 2], f32, kind="ExternalInput")
    jt_d = nc.dram_tensor("jt", [NS + 1, L], f32, kind="ExternalInput")
    id_d = nc.dram_tensor("ident", [B, B], f32, kind="ExternalInput")
    if has_blin:
        blin_d = nc.dram_tensor("blin_loc", [1, HDLOC], f32, kind="ExternalInput")
    if has_bout:
        bout_d = nc.dram_tensor("bout_loc", [1, OLOC], f32, kind="ExternalInput")
    out_d = nc.dram_tensor("out", [B, L, OLOC], f32, kind="ExternalOutput")

    zg_d = nc.dram_tensor("zgather", [H, B], f32r, kind="Internal", addr_space="Shared")

    with tile.TileContext(nc) as tc:
        with (
            tc.tile_pool(name="pers", bufs=1) as pers,
            tc.tile_pool(name="ztpool", bufs=2 * KT) as ztp,
            tc.tile_pool(name="psmall", bufs=2, space="PSUM") as psmall,
            tc.tile_pool(name="dram", bufs=1, space="DRAM") as dram,
        ):
            # persistent small tiles
            sb_wot = []
            for t in range(KT):
                wot = pers.tile([128, OLOC], f32r, tag=f"wot{t}", name=f"wot{t}")
                nc.sync.dma_start(wot[:], wo_d[128 * t:128 * (t + 1), :])
                sb_wot.append(wot)
            sb_ident = pers.tile([B, B], f32, tag="ident")
            nc.sync.dma_start(sb_ident[:], id_d[:])
            sb_jt = pers.tile([NS + 1, L], f32, tag="jt")
            nc.sync.dma_start(sb_jt[:], jt_d[:])
            sb_dx = pers.tile([B, NS, D], f32, tag="dx")
            sb_zsl = pers.tile([HLOC, B], f32, tag="zsl")
            sb_p = pers.tile([NS + 1, B * OLOC], f32, tag="P")
            if has_blin:
                sb_blin = pers.tile([1, HDLOC], f32, tag="blin")
                nc.sync.dma_start(sb_blin[:], blin_d[:])
            if has_bout:
                sb_bout = pers.tile([1, OLOC], f32, tag="bout")
                nc.sync.dma_start(sb_bout[:], bout_d[:])
            if has_blin or has_bout:
                sb_ones = pers.tile([1, B], f32, tag="ones")
                nc.vector.memset(sb_ones[:], 1.0)

            md_d = dram.tile([2, B * D], f32)
            p_d = dram.tile([NS + 1, B * OLOC], f32)
            zin_d = dram.tile([HLOC, B], f32r)

            # ---- setup scope (tiles die before the big loop) ---------------
            with tc.tile_pool(name="setup", bufs=1) as sp:
                sb_wg = sp.tile([L, 2], f32, tag="wg")
                nc.sync.dma_start(sb_wg[:], wg_d[:])
                sb_lbd = sp.tile([L, B, D], f32, tag="lbd")
                nc.sync.dma_start(sb_lbd[:], traj_d.ap().rearrange("b l d -> l b d"))

                lbd_flat = sb_lbd[:].rearrange("l b d -> l (b d)")
                sb_md = sp.tile([2, B * D], f32, tag="md")
                for c in range(B * D // 512):
                    ps_md = psmall.tile([2, 512], f32, tag="misc", name="ps_md")
                    nc.tensor.matmul(ps_md[:], sb_wg[:],
                                     lbd_flat[:, 512 * c:512 * (c + 1)])
                    nc.vector.tensor_copy(sb_md[:, 512 * c:512 * (c + 1)], ps_md[:])
                nc.sync.dma_start(md_d[:], sb_md[:])

                sb_m = sp.tile([B, D], f32, tag="m")
                sb_base = sp.tile([B, D], f32, tag="base")
                nc.sync.dma_start(sb_m[:], md_d[0, :].rearrange("(b d) -> b d", b=B))
                nc.sync.dma_start(sb_base[:], md_d[1, :].rearrange("(b d) -> b d", b=B))

                for k in range(NS):
                    nc.vector.scalar_tensor_tensor(
                        sb_dx[:, k, :], sb_m[:], float(mcoef[k]), sb_base[:],
                        op0=MUL, op1=ADD)

                # z0: augmented traj0^T (rows 0..63 = traj[:,0,:]^T, row 64 = 1)
                sb_t0 = sp.tile([D + 1, B], f32, tag="t0")
                nc.sync.dma_start(
                    sb_t0[:D, :], traj_d.ap().rearrange("b l d -> l d b")[0])
                nc.vector.memset(sb_t0[D:D + 1, :], 1.0)

                sb_wz0 = sp.tile([D + 1, H], f32, tag="wz0")
                nc.sync.dma_start(sb_wz0[:], wz0_d[:])
                sb_wz0l = sp.tile([D + 1, HLOC], f32, tag="wz0l")
                nc.sync.dma_start(sb_wz0l[:], wz0l_d[:])

                sb_zt = []
                for t in range(KT):
                    ps_z = psmall.tile([128, B], f32, tag="misc", name="ps_z")
                    nc.tensor.matmul(ps_z[:], sb_wz0[:, 128 * t:128 * (t + 1)],
                                     sb_t0[:])
                    zt = ztp.tile([128, B], f32r, tag="zt", name="zt0")
                    nc.vector.tensor_copy(zt[:], ps_z[:])
                    sb_zt.append(zt)

                ps_zs = psmall.tile([HLOC, B], f32, tag="misc", name="ps_zs")
                nc.tensor.matmul(ps_zs[:], sb_wz0l[:], sb_t0[:])
                nc.vector.tensor_copy(sb_zsl[:], ps_zs[:])

            # ---- main loop scope ------------------------------------------
            with (
                tc.tile_pool(name="wpool", bufs=KT) as wpool,
                tc.tile_pool(name="work", bufs=4) as work,
                tc.tile_pool(name="upool", bufs=2) as upool,
                tc.tile_pool(name="psf", bufs=2 * GRP, space="PSUM") as psf,
            ):
                sb_w = []
                for t in range(KT):
                    wtile = wpool.tile([128, HDLOC], f32r, tag="W", name=f"w{t}")
                    for cc in range(4):
                        sl = slice(1536 * cc, 1536 * (cc + 1))
                        nc.sync.dma_start(wtile[:, sl],
                                          wt_d[128 * t:128 * (t + 1), sl])
                    sb_w.append(wtile)

                def project(k):
                    ps_p = psmall.tile([B, OLOC], f32, tag="misc", name="ps_p")
                    if has_bout:
                        nc.tensor.matmul(ps_p[:], sb_ones[:], sb_bout[:],
                                         start=True, stop=False)
                    for t in range(KT):
                        nc.tensor.matmul(
                            ps_p[:], sb_zt[t][:],
                            sb_wot[t][:],
                            start=(t == 0 and not has_bout), stop=(t == KT - 1))
                    sb_pst = work.tile([B, OLOC], f32, tag="pstage", name="pst")
                    nc.vector.tensor_copy(sb_pst[:], ps_p[:])
                    nc.sync.dma_start(
                        p_d[k, :].rearrange("(b o) -> b o", b=B), sb_pst[:])

                for k in range(ns):
                    dx_k = sb_dx[:, k, :]
                    sb_u = upool.tile([B, HLOC], f32, tag="U", name="u")
                    for g in range(NT // GRP):
                        ps_list = []
                        for ni in range(GRP):
                            n = g * GRP + ni
                            ps_f = psf.tile([B, 512], f32, tag="f", name="ps_f")
                            if has_blin:
                                nc.tensor.matmul(
                                    ps_f[:], sb_ones[:],
                                    sb_blin[:, 512 * n:512 * (n + 1)],
                                    start=True, stop=False)
                            for t in range(KT):
                                nc.tensor.matmul(
                                    ps_f[:], sb_zt[t][:],
                                    sb_w[t][:, 512 * n:512 * (n + 1)],
                                    start=(t == 0 and not has_blin),
                                    stop=(t == KT - 1))
                            ps_list.append((n, ps_f))
                        for n, ps_f in ps_list:
                            tmp = work.tile([B, 8, D], f32, tag="tmp", name="tmp")
                            nc.vector.tensor_tensor(
                                tmp[:],
                                ps_f[:].rearrange("b (h d) -> b h d", h=8),
                                dx_k[:, None, :].to_broadcast((B, 8, D)),
                                MUL)
                            nc.vector.tensor_reduce(
                                sb_u[:, 8 * n:8 * (n + 1)], tmp[:],
                                axis=mybir.AxisListType.X, op=ADD)

                    project(k)

                    # transpose U -> (HLOC, B), update slice, gather
                    ps_ut = psmall.tile([HLOC, B], f32, tag="misc", name="ps_ut")
                    nc.tensor.transpose(ps_ut[:], sb_u[:], sb_ident[:])
                    nc.vector.scalar_tensor_tensor(
                        sb_zsl[:], ps_ut[:], float(dts[k]), sb_zsl[:],
                        op0=MUL, op1=ADD)
                    sb_zslr = work.tile([HLOC, B], f32r, tag="zslr", name="zslr")
                    nc.vector.tensor_copy(sb_zslr[:], sb_zsl[:])
                    nc.sync.dma_start(zin_d[:], sb_zslr[:])
                    nc.gpsimd.collective_compute(
                        "AllGather", mybir.AluOpType.bypass,
                        replica_groups=[list(range(NC))],
                        ins=[zin_d[:]], outs=[zg_d.ap()],
                    )
                    sb_zt = []
                    for t in range(KT):
                        zt = ztp.tile([128, B], f32r, tag="zt", name="zt")
                        nc.sync.dma_start(zt[:],
                                          zg_d.ap()[128 * t:128 * (t + 1), :])
                        sb_zt.append(zt)

                project(ns)

                # ---- final interp + output --------------------------------
                nc.sync.dma_start(sb_p[:], p_d[:])
                out_lbo = out_d.ap().rearrange("b l o -> l b o")
                BCH = 512 // OLOC  # batch elems per output chunk
                for c in range(B * OLOC // 512):
                    ps_o = psmall.tile([L, 512], f32, tag="misc", name="ps_o")
                    nc.tensor.matmul(ps_o[:], sb_jt[:],
                                     sb_p[:, 512 * c:512 * (c + 1)])
                    sb_o = work.tile([L, 512], f32, tag="tmp", name="sb_o")
                    nc.vector.tensor_copy(sb_o[:], ps_o[:])
                    nc.sync.dma_start(
                        out_lbo[:, BCH * c:BCH * (c + 1), :],
                        sb_o[:].rearrange("l (b o) -> l b o", o=OLOC))

    nc.compile()
    return nc


def traced_run_args(inputs):
    """Build (nc, in_maps) exactly as kernel() would — for profiling."""
    nc, in_maps = _prepare(inputs)
    return nc, in_maps


def _prepare(inputs):
    traj = np.ascontiguousarray(np.asarray(inputs["traj"], dtype=np.float32))
    W_lin = np.asarray(inputs["W_lin"], dtype=np.float32)
    b_lin = np.asarray(inputs["b_lin"], dtype=np.float32)
    W_out = np.asarray(inputs["W_out"], dtype=np.float32)
    b_out = np.asarray(inputs["b_out"], dtype=np.float32)
    W_z0 = np.asarray(inputs["W_z0"], dtype=np.float32)
    b_z0 = np.asarray(inputs["b_z0"], dtype=np.float32)

    dts, mcoef, wG, JT = _host_constants()
    has_blin = bool(np.any(b_lin))
    has_bout = bool(np.any(b_out))

    key = (has_blin, has_bout)
    if key not in _prog_cache:
        _prog_cache[key] = _build_program(dts, mcoef, has_blin, has_bout)
    nc = _prog_cache[key]

    wz0_aug = np.concatenate([W_z0.T, b_z0[None, :]], axis=0).astype(np.float32)
    ident = np.eye(B, dtype=np.float32)

    in_maps = []
    for i in range(NC):
        hsl = slice(HLOC * i, HLOC * (i + 1))
        osl = slice(OLOC * i, OLOC * (i + 1))
        m = dict(
            traj=traj,
            wt_loc=np.ascontiguousarray(
                W_lin[HLOC * D * i:HLOC * D * (i + 1), :].T),
            wz0_aug=wz0_aug,
            wz0l_aug=np.ascontiguousarray(wz0_aug[:, hsl]),
            wo_loc=np.ascontiguousarray(W_out[osl, :].T),
            wg=wG,
            jt=JT,
            ident=ident,
        )
        if has_blin:
            m["blin_loc"] = np.ascontiguousarray(
                b_lin[None, HLOC * D * i:HLOC * D * (i + 1)])
        if has_bout:
            m["bout_loc"] = np.ascontiguousarray(b_out[None, osl])
        in_maps.append(m)

    return nc, in_maps


def kernel(**inputs):
    from concourse.bass_utils import run_bass_kernel_spmd

    nc, in_maps = _prepare(inputs)
    res = run_bass_kernel_spmd(nc, in_maps, core_ids=list(range(NC)))
    return np.concatenate([r["out"] for r in res.results], axis=2)



# revision 10
# speedup vs baseline: 1.1648x; 1.1648x over previous
"""Neural CDE encoder kernel for 8 Trainium2 NeuronCores.

Math (from the reference):
  - Natural cubic spline on unit-spaced knots; Euler times t_k = 0.05*k for
    k=0..19 all lie in interval [0,1), so only interval-0 coefficients matter:
        dX(t) = (y1 - y0) + M1 * (t^2/2 - 1/6)
    with M1 = <w, y> for a constant weight vector w over L.
  - Euler: z_{k+1} = z_k + dt * einsum('bhd,bd->bh', reshape(z W^T + b), dX_k)
  - Output: project grid z's with W_out, then linearly interpolate via a
    constant (L x 21) matrix.

Sharding: tensor-parallel over H (96 rows of H per core; 6144 rows of W_lin).
W^T shard kept SBUF-resident in bf16 (768, 6144). Per step:
  - main matmul in bf16 with 2-way col-tiling (tile_position (0,0)/(0,64)):
    kt 0-2 accumulate into psum partitions 0-63, kt 3-5 into 64-127.
  - DVE contraction: tmp = psum * dX (bf16), segmented reduce over d, kt
    halves folded once per step.
  - z state kept in f32; AllGather of f32 z-slices each step; gathered z is
    cast to bf16 stationary tiles for the next matmul.
W_out sharded over O (32 cols/core, bf16); per-step projections staged via
ACT copies; final interp matmul writes (64, 128, 32) per core; host concats.
"""

import numpy as np

B, L, D, H, O = 64, 128, 64, 768, 256
NS = 20            # Euler steps
NC = 8             # cores
HLOC = H // NC     # 96
HDLOC = HLOC * D   # 6144
OLOC = O // NC     # 32
KT = H // 128      # 6 contraction tiles
NT = HDLOC // 512  # 12 moving chunks per step

_prog_cache = {}


def _host_constants():
    grid = (np.arange(NS + 1, dtype=np.float32) * np.float32(0.05)).astype(np.float32)
    grid[-1] = np.float32(1.0)
    dts = (grid[1:] - grid[:-1]).astype(np.float32)
    tk = grid[:-1].astype(np.float64)
    mcoef = (tk * tk / 2.0 - 1.0 / 6.0).astype(np.float32)

    # w over L such that M1 = <w, y>
    n = L - 2
    A = 4.0 * np.eye(n) + np.eye(n, k=1) + np.eye(n, k=-1)
    r0 = np.linalg.solve(A, np.eye(n)[:, 0])
    w = np.zeros(L, dtype=np.float64)
    w[0:n] += 6.0 * r0
    w[1:n + 1] += -12.0 * r0
    w[2:n + 2] += 6.0 * r0
    wG = np.zeros((L, 2), dtype=np.float32)
    wG[:, 0] = w.astype(np.float32)
    wG[1, 1] = 1.0
    wG[0, 1] = -1.0  # column 1 extracts base = y1 - y0

    # Interp matrix J (L, NS+1)
    ts = np.linspace(0.0, 1.0, L, dtype=np.float32)
    j = np.clip(np.searchsorted(grid, ts, side="right") - 1, 0, NS - 1)
    wl = ((ts - grid[j]) / (grid[j + 1] - grid[j])).astype(np.float32)
    J = np.zeros((L, NS + 1), dtype=np.float32)
    J[np.arange(L), j] += 1.0 - wl
    J[np.arange(L), j + 1] += wl
    return dts, mcoef, wG, J.T.copy()  # JT (21, 128)


def _build_program(dts, mcoef, has_blin, has_bout, ns=NS):
    import concourse.bacc as bacc
    import concourse.mybir as mybir
    import concourse.tile as tile

    f32 = mybir.dt.float32
    bf16 = mybir.dt.bfloat16
    ADD = mybir.AluOpType.add
    MUL = mybir.AluOpType.mult
    COPY = mybir.ActivationFunctionType.Copy

    nc = bacc.Bacc("TRN2", target_bir_lowering=False, debug=False, num_devices=NC)

    # ---- I/O -------------------------------------------------------------
    traj_d = nc.dram_tensor("traj", [B, L, D], f32, kind="ExternalInput")
    wt_d = nc.dram_tensor("wt_loc", [H, HDLOC], bf16, kind="ExternalInput")
    wz0_d = nc.dram_tensor("wz0_aug", [D + 1, H], f32, kind="ExternalInput")
    wz0l_d = nc.dram_tensor("wz0l_aug", [D + 1, HLOC], f32, kind="ExternalInput")
    wo_d = nc.dram_tensor("wo_loc", [H, OLOC], bf16, kind="ExternalInput")
    wg_d = nc.dram_tensor("wg", [L, 2], f32, kind="ExternalInput")
    jt_d = nc.dram_tensor("jt", [NS + 1, L], f32, kind="ExternalInput")
    id_d = nc.dram_tensor("ident", [B, B], f32, kind="ExternalInput")
    if has_blin:
        blin_d = nc.dram_tensor("blin_loc", [1, HDLOC], f32, kind="ExternalInput")
    if has_bout:
        bout_d = nc.dram_tensor("bout_loc", [1, OLOC], f32, kind="ExternalInput")
    out_d = nc.dram_tensor("out", [B, L, OLOC], f32, kind="ExternalOutput")

    zg_d = nc.dram_tensor("zgather", [H, B], f32, kind="Internal", addr_space="Shared")

    with tile.TileContext(nc) as tc:
        with (
            tc.tile_pool(name="pers", bufs=1) as pers,
            tc.tile_pool(name="ztpool", bufs=2) as ztp,
            tc.tile_pool(name="dram", bufs=1, space="DRAM") as dram,
        ):
            # persistent tiles
            sb_wot = pers.tile([128, KT * OLOC], bf16, tag="wot")  # (128, 6*32)
            for t in range(KT):
                nc.sync.dma_start(sb_wot[:, OLOC * t:OLOC * (t + 1)],
                                  wo_d[128 * t:128 * (t + 1), :])
            sb_ident = pers.tile([B, B], f32, tag="ident")
            nc.sync.dma_start(sb_ident[:], id_d[:])
            sb_jt = pers.tile([NS + 1, L], f32, tag="jt")
            nc.sync.dma_start(sb_jt[:], jt_d[:])
            sb_dx = pers.tile([128, NS, D], f32, tag="dx")   # duplicated rows
            sb_zsl = pers.tile([HLOC, B], f32, tag="zsl")
            sb_p = pers.tile([NS + 1, B * OLOC], f32, tag="P")
            if has_blin:
                sb_blin = pers.tile([1, HDLOC], f32, tag="blin")
                nc.sync.dma_start(sb_blin[:], blin_d[:])
            if has_bout:
                sb_bout = pers.tile([1, OLOC], f32, tag="bout")
                nc.sync.dma_start(sb_bout[:], bout_d[:])
            if has_blin or has_bout:
                sb_ones = pers.tile([1, B], f32, tag="ones")
                nc.vector.memset(sb_ones[:], 1.0)

            p_d = dram.tile([NS + 1, B * OLOC], f32)
            zin_d = dram.tile([HLOC, B], f32)

            # W^T shard, bf16, 6 kt-tiles as one (128, 6*6144) tile
            sb_w = pers.tile([128, KT * HDLOC], bf16, tag="W")
            for t in range(KT):
                for cc in range(4):
                    nc.sync.dma_start(
                        sb_w[:, HDLOC * t + 1536 * cc:HDLOC * t + 1536 * (cc + 1)],
                        wt_d[128 * t:128 * (t + 1),
                             1536 * cc:1536 * (cc + 1)])

            # gathered z as bf16 stationary tiles (128, 6, 64)
            md_d = dram.tile([2, B * D], f32)

            # ---- setup scope ----------------------------------------------
            with (
                tc.tile_pool(name="setup", bufs=1) as sp,
                tc.tile_pool(name="ps_set", bufs=2, space="PSUM") as ps_set,
            ):
                sb_wg = sp.tile([L, 2], f32, tag="wg")
                nc.sync.dma_start(sb_wg[:], wg_d[:])
                sb_lbd = sp.tile([L, B, D], f32, tag="lbd")
                nc.sync.dma_start(sb_lbd[:], traj_d.ap().rearrange("b l d -> l b d"))

                lbd_flat = sb_lbd[:].rearrange("l b d -> l (b d)")
                sb_md = sp.tile([2, B * D], f32, tag="md")
                for c in range(B * D // 512):
                    ps_md = ps_set.tile([128, 512], f32, tag="pset", name="ps_md")
                    nc.tensor.matmul(ps_md[0:2, :], sb_wg[:],
                                     lbd_flat[:, 512 * c:512 * (c + 1)],
                                     start=True, stop=True)
                    nc.vector.tensor_copy(sb_md[:, 512 * c:512 * (c + 1)],
                                          ps_md[0:2, :])
                nc.sync.dma_start(md_d[:], sb_md[:])

                # m, base duplicated to 128 partitions
                sb_m = sp.tile([128, D], f32, tag="m")
                sb_base = sp.tile([128, D], f32, tag="base")
                for half in range(2):
                    nc.sync.dma_start(
                        sb_m[64 * half:64 * (half + 1), :],
                        md_d[0, :].rearrange("(b d) -> b d", b=B))
                    nc.sync.dma_start(
                        sb_base[64 * half:64 * (half + 1), :],
                        md_d[1, :].rearrange("(b d) -> b d", b=B))

                for k in range(NS):
                    nc.vector.scalar_tensor_tensor(
                        sb_dx[:, k, :], sb_m[:], float(mcoef[k]), sb_base[:],
                        op0=MUL, op1=ADD)

                # z0: augmented traj0^T
                sb_t0 = sp.tile([D + 1, B], f32, tag="t0")
                nc.sync.dma_start(
                    sb_t0[:D, :], traj_d.ap().rearrange("b l d -> l d b")[0])
                nc.vector.memset(sb_t0[D:D + 1, :], 1.0)

                sb_wz0 = sp.tile([D + 1, H], f32, tag="wz0")
                nc.sync.dma_start(sb_wz0[:], wz0_d[:])
                sb_wz0l = sp.tile([D + 1, HLOC], f32, tag="wz0l")
                nc.sync.dma_start(sb_wz0l[:], wz0l_d[:])

                sb_zt = ztp.tile([128, KT, B], bf16, tag="zt", name="zt0")
                for t in range(KT):
                    ps_z = ps_set.tile([128, 512], f32, tag="pset", name="ps_z")
                    nc.tensor.matmul(ps_z[:, 0:B], sb_wz0[:, 128 * t:128 * (t + 1)],
                                     sb_t0[:], start=True, stop=True)
                    nc.vector.tensor_copy(sb_zt[:, t, :], ps_z[:, 0:B])

                ps_zs = ps_set.tile([128, 512], f32, tag="pset", name="ps_zs")
                nc.tensor.matmul(ps_zs[0:HLOC, 0:B], sb_wz0l[:], sb_t0[:],
                                 start=True, stop=True)
                nc.vector.tensor_copy(sb_zsl[:], ps_zs[0:HLOC, 0:B])

            # ---- main loop -------------------------------------------------
            with (
                tc.tile_pool(name="work", bufs=3) as work,
                tc.tile_pool(name="upool", bufs=2) as upool,
                tc.tile_pool(name="psf", bufs=4, space="PSUM") as psf,
                tc.tile_pool(name="pst", bufs=2, space="PSUM") as pst,
                tc.tile_pool(name="psp", bufs=2, space="PSUM") as psp,
            ):
                def project(k, zt):
                    # psum (64, 32) <- zt^T @ wot ; staged to sb_p row k via ACT
                    ps_p = psp.tile([128, 512], f32, tag="pp", name="ps_p")
                    if has_bout:
                        nc.tensor.matmul(ps_p[0:B, 0:OLOC], sb_ones[:], sb_bout[:],
                                         start=True, stop=False)
                    for t in range(KT):
                        nc.tensor.matmul(
                            ps_p[0:B, 0:OLOC], zt[:, t, :],
                            sb_wot[:, OLOC * t:OLOC * (t + 1)],
                            start=(t == 0 and not has_bout), stop=(t == KT - 1))
                    sb_pst = work.tile([B, OLOC], f32, tag="pstage", name="pst")
                    nc.scalar.activation(sb_pst[:], ps_p[0:B, 0:OLOC], COPY)
                    nc.sync.dma_start(
                        p_d[k, :].rearrange("(b o) -> b o", b=B), sb_pst[:])

                project(0, sb_zt)

                for k in range(ns):
                    dx_k = sb_dx[:, k, :]
                    sb_u = upool.tile([128, HLOC], f32, tag="U", name="u")
                    for n in range(NT):
                        ps_f = psf.tile([128, 512], f32, tag="f", name="ps_f")
                        for ti in range(3):
                            if has_blin:
                                raise NotImplementedError
                            nc.tensor.matmul(
                                ps_f[0:64, :], sb_zt[:, ti, :],
                                sb_w[:, HDLOC * ti + 512 * n:
                                     HDLOC * ti + 512 * (n + 1)],
                                start=(ti == 0), stop=(ti == 2),
                                tile_position=(0, 0))
                            nc.tensor.matmul(
                                ps_f[64:128, :], sb_zt[:, ti + 3, :],
                                sb_w[:, HDLOC * (ti + 3) + 512 * n:
                                     HDLOC * (ti + 3) + 512 * (n + 1)],
                                start=(ti == 0), stop=(ti == 2),
                                tile_position=(0, 64))
                        tmp = work.tile([128, 512], bf16, tag="tmp", name="tmp")
                        nc.vector.tensor_tensor(
                            tmp[:].rearrange("p (h d) -> p h d", d=D),
                            ps_f[:].rearrange("p (h d) -> p h d", d=D),
                            dx_k[:, None, :].to_broadcast((128, 8, D)),
                            MUL)
                        nc.vector.tensor_reduce(
                            sb_u[:, 8 * n:8 * (n + 1)],
                            tmp[:].rearrange("p (h d) -> p h d", d=D),
                            axis=mybir.AxisListType.X, op=ADD)

                    # fold kt halves: (64, 96)
                    sb_u2 = work.tile([B, HLOC], f32, tag="u2", name="u2")
                    nc.vector.tensor_copy(sb_u2[:], sb_u[64:128, :])
                    sb_uf = work.tile([B, HLOC], f32, tag="uf", name="uf")
                    nc.vector.tensor_tensor(
                        sb_uf[:], sb_u[0:64, :], sb_u2[:], ADD)

                    # transpose U -> (96, 64), update z slice, send + gather
                    ps_ut = pst.tile([128, 512], f32, tag="ut", name="ps_ut")
                    nc.tensor.transpose(ps_ut[0:HLOC, 0:B], sb_uf[:], sb_ident[:])
                    nc.vector.scalar_tensor_tensor(
                        sb_zsl[:], ps_ut[0:HLOC, 0:B], float(dts[k]), sb_zsl[:],
                        op0=MUL, op1=ADD)
                    nc.sync.dma_start(zin_d[:], sb_zsl[:])
                    nc.gpsimd.collective_compute(
                        "AllGather", mybir.AluOpType.bypass,
                        replica_groups=[list(range(NC))],
                        ins=[zin_d[:]], outs=[zg_d.ap()],
                    )
                    sb_zg = work.tile([128, KT * B], f32, tag="zg", name="zg")
                    nc.sync.dma_start(
                        sb_zg[:].rearrange("p (t b) -> p t b", t=KT),
                        zg_d.ap().rearrange("(t p) b -> p t b", p=128))
                    sb_zt = ztp.tile([128, KT, B], bf16, tag="zt", name="zt")
                    nc.vector.tensor_copy(
                        sb_zt[:], sb_zg[:].rearrange("p (t b) -> p t b", t=KT))

                    project(k + 1, sb_zt)

                # ---- final interp + output --------------------------------
                nc.sync.dma_start(sb_p[:], p_d[:])
                out_lbo = out_d.ap().rearrange("b l o -> l b o")
                BCH = 512 // OLOC  # batch elems per output chunk
                for c in range(B * OLOC // 512):
                    ps_o = psp.tile([128, 512], f32, tag="pp", name="ps_o")
                    nc.tensor.matmul(ps_o[0:L, :], sb_jt[:],
                                     sb_p[:, 512 * c:512 * (c + 1)],
                                     start=True, stop=True)
                    sb_o = work.tile([L, 512], f32, tag="outstage", name="sb_o")
                    nc.scalar.activation(sb_o[:], ps_o[0:L, :], COPY)
                    nc.sync.dma_start(
                        out_lbo[:, BCH * c:BCH * (c + 1), :],
                        sb_o[:].rearrange("l (b o) -> l b o", o=OLOC))

    nc.compile()
    return nc


def _prepare(inputs):
    import ml_dtypes

    traj = np.ascontiguousarray(np.asarray(inputs["traj"], dtype=np.float32))
    W_lin = np.asarray(inputs["W_lin"], dtype=np.float32)
    b_lin = np.asarray(inputs["b_lin"], dtype=np.float32)
    W_out = np.asarray(inputs["W_out"], dtype=np.float32)
    b_out = np.asarray(inputs["b_out"], dtype=np.float32)
    W_z0 = np.asarray(inputs["W_z0"], dtype=np.float32)
    b_z0 = np.asarray(inputs["b_z0"], dtype=np.float32)

    dts, mcoef, wG, JT = _host_constants()
    has_blin = bool(np.any(b_lin))
    has_bout = bool(np.any(b_out))
    if has_blin:
        raise NotImplementedError("b_lin != 0 not supported in fast path")

    key = (has_blin, has_bout)
    if key not in _prog_cache:
        _prog_cache[key] = _build_program(dts, mcoef, has_blin, has_bout)
    nc = _prog_cache[key]

    wz0_aug = np.concatenate([W_z0.T, b_z0[None, :]], axis=0).astype(np.float32)
    ident = np.eye(B, dtype=np.float32)
    WT_bf = np.ascontiguousarray(W_lin.T).astype(ml_dtypes.bfloat16)  # (768, 49152)
    WO_bf = np.ascontiguousarray(W_out.T).astype(ml_dtypes.bfloat16)  # (768, 256)

    in_maps = []
    for i in range(NC):
        hsl = slice(HLOC * i, HLOC * (i + 1))
        osl = slice(OLOC * i, OLOC * (i + 1))
        m = dict(
            traj=traj,
            wt_loc=np.ascontiguousarray(
                WT_bf[:, HLOC * D * i:HLOC * D * (i + 1)]),
            wz0_aug=wz0_aug,
            wz0l_aug=np.ascontiguousarray(wz0_aug[:, hsl]),
            wo_loc=np.ascontiguousarray(WO_bf[:, osl]),
            wg=wG,
            jt=JT,
            ident=ident,
        )
        if has_bout:
            m["bout_loc"] = np.ascontiguousarray(b_out[None, osl])
        in_maps.append(m)

    return nc, in_maps


def traced_run_args(inputs):
    """Build (nc, in_maps) exactly as kernel() would — for profiling."""
    return _prepare(inputs)


def kernel(**inputs):
    from concourse.bass_utils import run_bass_kernel_spmd

    nc, in_maps = _prepare(inputs)
    res = run_bass_kernel_spmd(nc, in_maps, core_ids=list(range(NC)))
    return np.concatenate([r["out"] for r in res.results], axis=2)
